# revision 10
# baseline (speedup 1.0000x reference)
"""GCN layer kernel for Trainium2, 8 NeuronCores.

Computation (see reference): out = relu(segment_sum(vals * (X @ W)[cols], rows))
with X = concat(u_f, v_f) [100000, 128], 1.6M edges.

Strategy:
  - Shard destination nodes across the 8 cores (12500 rows each).
  - Dense transform: each core computes its 12500-row shard of
    node_f = X @ W on the TensorEngine (fp32), casts to fp16, and an
    AllGather replicates the full node_f [100000, 128] fp16 into every
    core's DRAM.
  - Edges are bucketed on the host by (core, superblock of 4 dest blocks,
    25000-row source window, dest block), each (block, window) subcell padded
    to a multiple of 128 edges (padding uniform across cores so all cores run
    the same program).  Per (superblock, window) one dma_gather fetches all
    source rows (int16 indices relative to the window start) into
    [128 edges x 128 feat] fp16 tiles in SBUF.
  - Segment sum via selection-matrix matmuls: for each 128-edge tile,
    SelT[e, d] = vals[e] * (rows_in_block[e] == d) is built in one DVE
    tensor_scalar(is_equal, mult) op; PSUM accumulates SelT.T @ G over all of
    a block's tiles.  ReLU on the way out via the scalar engine.

kernel(**inputs) takes full unsharded numpy inputs and returns the full
[100000, 128] float32 output.
"""

import math

import numpy as np

import concourse.tile as tile
from concourse import bacc, mybir
from concourse.bass_utils import run_bass_kernel_spmd

P = 128
N_CORES = 8
N_NODES = 100000
D = 128
DESTS_PER_CORE = N_NODES // N_CORES  # 12500
N_BLOCKS = math.ceil(DESTS_PER_CORE / P)  # 98 blocks of 128 dests (last 84)
N_WINDOWS = 4
WINDOW = 25000  # source rows per gather window (int16-addressable)
SUPER = 4  # dest blocks per gather superblock
F32 = mybir.dt.float32
F16 = mybir.dt.float16
I16 = mybir.dt.int16


def _superblocks():
    return [
        list(range(s, min(s + SUPER, N_BLOCKS))) for s in range(0, N_BLOCKS, SUPER)
    ]


def _build_program(cell_sizes, nt_total, idx_cols_total):
    """Build the SPMD Bass program (identical across cores).

    cell_sizes[b][q]: padded edge count of (dest block b, window q), multiple
    of 128, same for all cores.
    """
    nc = bacc.Bacc(
        "TRN2",
        target_bir_lowering=False,
        debug=False,
        num_swdge_queues=4,
        num_devices=N_CORES,
    )

    x_t = nc.dram_tensor("x_t", [P, DESTS_PER_CORE], F32, kind="ExternalInput")
    w_in = nc.dram_tensor("w", [P, D], F32, kind="ExternalInput")
    rows_in = nc.dram_tensor("rows", [P, nt_total], F16, kind="ExternalInput")
    vals_in = nc.dram_tensor("vals", [P, nt_total], F32, kind="ExternalInput")
    idxs_in = nc.dram_tensor("idxs", [P, idx_cols_total], I16, kind="ExternalInput")
    iota_in = nc.dram_tensor("iota", [P, P], F16, kind="ExternalInput")
    out = nc.dram_tensor("out", [DESTS_PER_CORE, D], F32, kind="ExternalOutput")

    sblocks = _superblocks()
    max_sb_tiles = max(
        sum(cell_sizes[b][q] for b in sb) // P for sb in sblocks for q in range(N_WINDOWS)
    )

    with tile.TileContext(nc) as tc:
        with (
            tc.tile_pool(name="const", bufs=1) as const_pool,
            tc.tile_pool(name="dram", bufs=1, space="DRAM") as dram_pool,
            tc.tile_pool(name="gpool", bufs=8) as g_pool,
            tc.tile_pool(name="selpool", bufs=10) as sel_pool,
            tc.tile_pool(name="outstage", bufs=6) as out_pool,
            tc.tile_pool(name="psum", bufs=4, space="PSUM") as psum_pool,
        ):
            # ---- persistent SBUF state ----
            w_sb = const_pool.tile([P, D], F32, tag="w")
            nc.sync.dma_start(w_sb[:], w_in[:])
            iota_sb = const_pool.tile([P, P], F16, tag="iota")
            nc.sync.dma_start(iota_sb[:], iota_in[:])
            rows_sb = const_pool.tile([P, nt_total], F16, tag="rows")
            nc.sync.dma_start(rows_sb[:], rows_in[:])
            vals_sb = const_pool.tile([P, nt_total], F32, tag="vals")
            nc.sync.dma_start(vals_sb[:], vals_in[:])
            idxs_sb = const_pool.tile([P, idx_cols_total], I16, tag="idxs")
            nc.sync.dma_start(idxs_sb[:], idxs_in[:])
            xfull_sb = const_pool.tile([P, DESTS_PER_CORE], F32, tag="xfull")
            nc.sync.dma_start(xfull_sb[:], x_t[:])

            # ---- dense phase: nf_local = (x_shard @ W) cast to fp16 ----
            nf_local = dram_pool.tile([DESTS_PER_CORE, D], F16)
            nf_full = dram_pool.tile([N_NODES, D], F16, addr_space="Shared")
            for t in range(N_BLOCKS):
                lo = t * P
                pm = min(P, DESTS_PER_CORE - lo)
                ps = psum_pool.tile([P, D], F32, tag="dense_ps", bufs=2)
                nc.tensor.matmul(
                    out=ps[:pm, :],
                    lhsT=xfull_sb[:, lo : lo + pm],
                    rhs=w_sb[:],
                    start=True,
                    stop=True,
                )
                stage = out_pool.tile([P, D], F16, tag="dense_out")
                nc.scalar.activation(
                    out=stage[:pm, :],
                    in_=ps[:pm, :],
                    func=mybir.ActivationFunctionType.Copy,
                )
                nc.sync.dma_start(nf_local[lo : lo + pm, :], stage[:pm, :])

            nc.gpsimd.collective_compute(
                "AllGather",
                mybir.AluOpType.bypass,
                replica_groups=[list(range(N_CORES))],
                ins=[nf_local[:].opt()],
                outs=[nf_full[:].opt()],
            )

            # ---- edge phase ----
            gq = 0  # gather queue rotation
            tile_pos = 0  # running tile index into rows/vals (matmul order)
            idx_pos = 0  # running int16 column index into idxs (gather order)
            for sb in sblocks:
                # one gather per (superblock, window)
                g_tiles = {}  # q -> (tile handle, {b: tile offset})
                for q in range(N_WINDOWS):
                    cell_n = sum(cell_sizes[b][q] for b in sb)
                    if cell_n == 0:
                        continue
                    n_tiles = cell_n // P
                    g_sb = g_pool.tile([P, max_sb_tiles, P], F16, tag="g")
                    nc.gpsimd.dma_gather(
                        g_sb[:, :n_tiles, :],
                        nf_full[q * WINDOW : (q + 1) * WINDOW, :],
                        idxs_sb[:, idx_pos : idx_pos + cell_n // 16],
                        cell_n,
                        cell_n,
                        D,
                        single_packet=False,
                        queue_num=gq,
                    )
                    gq = (gq + 1) % 4
                    idx_pos += cell_n // 16
                    offs = {}
                    off = 0
                    for b in sb:
                        offs[b] = off
                        off += cell_sizes[b][q] // P
                    g_tiles[q] = (g_sb, offs)

                for b in sb:
                    bs = min(P, DESTS_PER_CORE - b * P)
                    acc = psum_pool.tile([P, D], F32, tag="acc", bufs=6)
                    block_tiles = sum(cell_sizes[b]) // P
                    done = 0
                    for q in range(N_WINDOWS):
                        n_tiles = cell_sizes[b][q] // P
                        if n_tiles == 0:
                            continue
                        g_sb, offs = g_tiles[q]
                        for t in range(n_tiles):
                            sel = sel_pool.tile([P, P], F16, tag="sel")
                            nc.vector.scalar_tensor_tensor(
                                out=sel[:],
                                in0=iota_sb[:],
                                scalar=rows_sb[:, tile_pos : tile_pos + 1],
                                in1=vals_sb[:, tile_pos : tile_pos + 1].to_broadcast(
                                    [P, P]
                                ),
                                op0=mybir.AluOpType.is_equal,
                                op1=mybir.AluOpType.mult,
                            )
                            nc.tensor.matmul(
                                out=acc[:],
                                lhsT=sel[:],
                                rhs=g_sb[:, offs[b] + t, :],
                                start=(done == 0),
                                stop=(done == block_tiles - 1),
                            )
                            done += 1
                            tile_pos += 1
                    stage = out_pool.tile([P, D], F32, tag="edge_out")
                    nc.scalar.activation(
                        out=stage[:bs, :],
                        in_=acc[:bs, :],
                        func=mybir.ActivationFunctionType.Relu,
                    )
                    nc.sync.dma_start(out[b * P : b * P + bs, :], stage[:bs, :])

    nc.compile()
    return nc


_CACHE = {}


def _prepare(u_f, v_f, adj_rows, adj_cols, adj_vals):
    """Host-side sharding: bucket edges by (core, superblock, window, block),
    pad (block, window) subcells to multiples of 128 (uniform across cores),
    and lay out per-core rows/vals/idx arrays in the SBUF tile layouts.

    Gather (idxs) order: superblock -> window -> block -> edges.
    Matmul (rows/vals) order: superblock -> block -> window -> edges.
    """
    rows = np.asarray(adj_rows, dtype=np.int64)
    cols = np.asarray(adj_cols, dtype=np.int64)
    vals = np.asarray(adj_vals, dtype=np.float32)

    core_of = rows // DESTS_PER_CORE
    blk_of = (rows % DESTS_PER_CORE) // P
    win_of = cols // WINDOW

    key = (core_of * N_BLOCKS + blk_of) * N_WINDOWS + win_of
    order = np.argsort(key, kind="stable")
    rows_s = rows[order]
    cols_s = cols[order]
    vals_s = vals[order]

    n_cells_total = N_CORES * N_BLOCKS * N_WINDOWS
    counts = np.bincount(key[order], minlength=n_cells_total).reshape(
        N_CORES, N_BLOCKS, N_WINDOWS
    )
    starts = np.zeros(n_cells_total + 1, dtype=np.int64)
    np.cumsum(counts.reshape(-1), out=starts[1:])

    max_counts = counts.max(axis=0)  # [N_BLOCKS, N_WINDOWS]
    cell_sizes = (np.ceil(max_counts / P).astype(np.int64) * P).tolist()
    for b in range(N_BLOCKS):
        if sum(cell_sizes[b]) == 0:
            cell_sizes[b][0] = P  # keep PSUM written for empty blocks

    total_padded = sum(sum(cs) for cs in cell_sizes)
    nt_total = total_padded // P
    idx_cols_total = total_padded // 16
    sblocks = _superblocks()

    per_core = []
    for c in range(N_CORES):
        rows_t = np.zeros((P, nt_total), np.float16)
        vals_t = np.zeros((P, nt_total), np.float32)
        idxs_t = np.zeros((P, idx_cols_total), np.int16)
        tile_pos = 0
        idx_pos = 0

        def cell_edges(b, q):
            ci = (c * N_BLOCKS + b) * N_WINDOWS + q
            s = starts[ci]
            e = s + counts[c, b, q]
            return rows_s[s:e], cols_s[s:e], vals_s[s:e]

        for sb in sblocks:
            # gather (idxs) order: window -> block
            for q in range(N_WINDOWS):
                for b in sb:
                    cell_n = cell_sizes[b][q]
                    if cell_n == 0:
                        continue
                    _, cc, _ = cell_edges(b, q)
                    ii = np.zeros(cell_n, np.int16)
                    ii[: len(cc)] = (cc - q * WINDOW).astype(np.int16)
                    wrapped = ii.reshape(cell_n // 16, 16).T
                    for g in range(8):
                        idxs_t[
                            16 * g : 16 * (g + 1), idx_pos : idx_pos + cell_n // 16
                        ] = wrapped
                    idx_pos += cell_n // 16
            # matmul (rows/vals) order: block -> window
            for b in sb:
                for q in range(N_WINDOWS):
                    cell_n = cell_sizes[b][q]
                    if cell_n == 0:
                        continue
                    rr, _, vv = cell_edges(b, q)
                    r = np.zeros(cell_n, np.float32)
                    v = np.zeros(cell_n, np.float32)
                    r[: len(rr)] = (rr - c * DESTS_PER_CORE - b * P).astype(np.float32)
                    v[: len(vv)] = vv
                    nt = cell_n // P
                    rows_t[:, tile_pos : tile_pos + nt] = r.reshape(nt, P).T
                    vals_t[:, tile_pos : tile_pos + nt] = v.reshape(nt, P).T
                    tile_pos += nt
        per_core.append((rows_t, vals_t, idxs_t))

    x_full = np.concatenate(
        [np.asarray(u_f, np.float32), np.asarray(v_f, np.float32)], axis=0
    )
    iota = np.broadcast_to(np.arange(P, dtype=np.float16), (P, P)).copy()
    in_maps = []
    for c in range(N_CORES):
        rows_t, vals_t, idxs_t = per_core[c]
        in_maps.append(
            {
                "x_t": np.ascontiguousarray(
                    x_full[c * DESTS_PER_CORE : (c + 1) * DESTS_PER_CORE, :].T
                ),
                "w": None,  # filled by caller
                "rows": rows_t,
                "vals": vals_t,
                "idxs": idxs_t,
                "iota": iota,
            }
        )
    return cell_sizes, nt_total, idx_cols_total, in_maps


def kernel(u_f, v_f, adj_rows, adj_cols, adj_vals, weight):
    w = np.asarray(weight, np.float32)
    cell_sizes, nt_total, idx_cols_total, in_maps = _prepare(
        u_f, v_f, adj_rows, adj_cols, adj_vals
    )
    for m in in_maps:
        m["w"] = w

    cache_key = (nt_total, idx_cols_total, tuple(tuple(cs) for cs in cell_sizes))
    if cache_key not in _CACHE:
        _CACHE.clear()
        _CACHE[cache_key] = _build_program(cell_sizes, nt_total, idx_cols_total)
    nc = _CACHE[cache_key]

    # The axon-tunneled device occasionally reports a transient
    # NRT_EXEC_UNIT_UNRECOVERABLE from a previous crashed run; a retry runs
    # on the freshly-reset device.
    last_err = None
    for _ in range(4):
        try:
            res = run_bass_kernel_spmd(nc, in_maps, core_ids=list(range(N_CORES)))
            break
        except Exception as e:  # noqa: BLE001
            last_err = e
    else:
        raise last_err
    return np.concatenate([res.results[c]["out"] for c in range(N_CORES)], axis=0)


# revision 11
# speedup vs baseline: 1.0035x; 1.0035x over previous
"""GCN layer kernel for Trainium2, 8 NeuronCores.

Computation (see reference): out = relu(segment_sum(vals * (X @ W)[cols], rows))
with X = concat(u_f, v_f) [100000, 128], 1.6M edges.

Strategy:
  - Shard destination nodes across the 8 cores (12500 rows each).
  - Dense transform: each core computes its 12500-row shard of
    node_f = X @ W on the TensorEngine (fp32), casts to fp16, and an
    AllGather replicates the full node_f [100000, 128] fp16 into every
    core's DRAM.
  - Edges are bucketed on the host by (core, superblock of 4 dest blocks,
    25000-row source window, dest block), each (block, window) subcell padded
    to a multiple of 128 edges (padding uniform across cores so all cores run
    the same program).  Per (superblock, window) one dma_gather fetches all
    source rows (int16 indices relative to the window start) into
    [128 edges x 128 feat] fp16 tiles in SBUF.
  - Segment sum via selection-matrix matmuls: for each 128-edge tile,
    SelT[e, d] = vals[e] * (rows_in_block[e] == d) is built in one DVE
    tensor_scalar(is_equal, mult) op; PSUM accumulates SelT.T @ G over all of
    a block's tiles.  ReLU on the way out via the scalar engine.

kernel(**inputs) takes full unsharded numpy inputs and returns the full
[100000, 128] float32 output.
"""

import math

import numpy as np

import concourse.tile as tile
from concourse import bacc, mybir
from concourse.bass_utils import run_bass_kernel_spmd

P = 128
N_CORES = 8
N_NODES = 100000
D = 128
DESTS_PER_CORE = N_NODES // N_CORES  # 12500
N_BLOCKS = math.ceil(DESTS_PER_CORE / P)  # 98 blocks of 128 dests (last 84)
N_WINDOWS = 4
WINDOW = 25000  # source rows per gather window (int16-addressable)
SUPER = 4  # dest blocks per gather superblock
F32 = mybir.dt.float32
F16 = mybir.dt.float16
I16 = mybir.dt.int16


def _superblocks():
    return [
        list(range(s, min(s + SUPER, N_BLOCKS))) for s in range(0, N_BLOCKS, SUPER)
    ]


def _build_program(cell_sizes, nt_total, idx_cols_total):
    """Build the SPMD Bass program (identical across cores).

    cell_sizes[b][q]: padded edge count of (dest block b, window q), multiple
    of 128, same for all cores.
    """
    nc = bacc.Bacc(
        "TRN2",
        target_bir_lowering=False,
        debug=False,
        num_swdge_queues=4,
        num_devices=N_CORES,
        dynamic_dma_scratch_size=49152,
    )

    x_t = nc.dram_tensor("x_t", [P, DESTS_PER_CORE], F32, kind="ExternalInput")
    w_in = nc.dram_tensor("w", [P, D], F32, kind="ExternalInput")
    rows_in = nc.dram_tensor("rows", [P, nt_total], F16, kind="ExternalInput")
    vals_in = nc.dram_tensor("vals", [P, nt_total], F32, kind="ExternalInput")
    idxs_in = nc.dram_tensor("idxs", [P, idx_cols_total], I16, kind="ExternalInput")
    iota_in = nc.dram_tensor("iota", [P, P], F16, kind="ExternalInput")
    out = nc.dram_tensor("out", [DESTS_PER_CORE, D], F32, kind="ExternalOutput")

    sblocks = _superblocks()
    max_sb_tiles = max(
        sum(cell_sizes[b][q] for b in sb) // P for sb in sblocks for q in range(N_WINDOWS)
    )

    with tile.TileContext(nc) as tc:
        with (
            tc.tile_pool(name="const", bufs=1) as const_pool,
            tc.tile_pool(name="dram", bufs=1, space="DRAM") as dram_pool,
            tc.tile_pool(name="gpool", bufs=8) as g_pool,
            tc.tile_pool(name="selpool", bufs=10) as sel_pool,
            tc.tile_pool(name="outstage", bufs=6) as out_pool,
            tc.tile_pool(name="psum", bufs=4, space="PSUM") as psum_pool,
        ):
            # ---- persistent SBUF state ----
            w_sb = const_pool.tile([P, D], F32, tag="w")
            nc.sync.dma_start(w_sb[:], w_in[:])
            iota_sb = const_pool.tile([P, P], F16, tag="iota")
            nc.sync.dma_start(iota_sb[:], iota_in[:])
            rows_sb = const_pool.tile([P, nt_total], F16, tag="rows")
            nc.sync.dma_start(rows_sb[:], rows_in[:])
            vals_sb = const_pool.tile([P, nt_total], F32, tag="vals")
            nc.sync.dma_start(vals_sb[:], vals_in[:])
            idxs_sb = const_pool.tile([P, idx_cols_total], I16, tag="idxs")
            nc.sync.dma_start(idxs_sb[:], idxs_in[:])
            xfull_sb = const_pool.tile([P, DESTS_PER_CORE], F32, tag="xfull")
            nc.sync.dma_start(xfull_sb[:], x_t[:])

            # ---- dense phase: nf_local = (x_shard @ W) cast to fp16 ----
            nf_local = dram_pool.tile([DESTS_PER_CORE, D], F16)
            nf_full = dram_pool.tile([N_NODES, D], F16, addr_space="Shared")
            for t in range(N_BLOCKS):
                lo = t * P
                pm = min(P, DESTS_PER_CORE - lo)
                ps = psum_pool.tile([P, D], F32, tag="dense_ps", bufs=2)
                nc.tensor.matmul(
                    out=ps[:pm, :],
                    lhsT=xfull_sb[:, lo : lo + pm],
                    rhs=w_sb[:],
                    start=True,
                    stop=True,
                )
                stage = out_pool.tile([P, D], F16, tag="dense_out")
                nc.scalar.activation(
                    out=stage[:pm, :],
                    in_=ps[:pm, :],
                    func=mybir.ActivationFunctionType.Copy,
                )
                nc.sync.dma_start(nf_local[lo : lo + pm, :], stage[:pm, :])

            nc.gpsimd.collective_compute(
                "AllGather",
                mybir.AluOpType.bypass,
                replica_groups=[list(range(N_CORES))],
                ins=[nf_local[:].opt()],
                outs=[nf_full[:].opt()],
            )

            # ---- edge phase ----
            gq = 0  # gather queue rotation
            tile_pos = 0  # running tile index into rows/vals (matmul order)
            idx_pos = 0  # running int16 column index into idxs (gather order)
            for sb in sblocks:
                # one gather per (superblock, window)
                g_tiles = {}  # q -> (tile handle, {b: tile offset})
                for q in range(N_WINDOWS):
                    cell_n = sum(cell_sizes[b][q] for b in sb)
                    if cell_n == 0:
                        continue
                    n_tiles = cell_n // P
                    g_sb = g_pool.tile([P, max_sb_tiles, P], F16, tag="g")
                    nc.gpsimd.dma_gather(
                        g_sb[:, :n_tiles, :],
                        nf_full[q * WINDOW : (q + 1) * WINDOW, :],
                        idxs_sb[:, idx_pos : idx_pos + cell_n // 16],
                        cell_n,
                        cell_n,
                        D,
                        single_packet=False,
                        queue_num=gq,
                    )
                    gq = (gq + 1) % 4
                    idx_pos += cell_n // 16
                    offs = {}
                    off = 0
                    for b in sb:
                        offs[b] = off
                        off += cell_sizes[b][q] // P
                    g_tiles[q] = (g_sb, offs)

                for b in sb:
                    bs = min(P, DESTS_PER_CORE - b * P)
                    acc = psum_pool.tile([P, D], F32, tag="acc", bufs=6)
                    block_tiles = sum(cell_sizes[b]) // P
                    done = 0
                    for q in range(N_WINDOWS):
                        n_tiles = cell_sizes[b][q] // P
                        if n_tiles == 0:
                            continue
                        g_sb, offs = g_tiles[q]
                        for t in range(n_tiles):
                            sel = sel_pool.tile([P, P], F16, tag="sel")
                            nc.vector.scalar_tensor_tensor(
                                out=sel[:],
                                in0=iota_sb[:],
                                scalar=rows_sb[:, tile_pos : tile_pos + 1],
                                in1=vals_sb[:, tile_pos : tile_pos + 1].to_broadcast(
                                    [P, P]
                                ),
                                op0=mybir.AluOpType.is_equal,
                                op1=mybir.AluOpType.mult,
                            )
                            nc.tensor.matmul(
                                out=acc[:],
                                lhsT=sel[:],
                                rhs=g_sb[:, offs[b] + t, :],
                                start=(done == 0),
                                stop=(done == block_tiles - 1),
                            )
                            done += 1
                            tile_pos += 1
                    stage = out_pool.tile([P, D], F32, tag="edge_out")
                    nc.scalar.activation(
                        out=stage[:bs, :],
                        in_=acc[:bs, :],
                        func=mybir.ActivationFunctionType.Relu,
                    )
                    nc.sync.dma_start(out[b * P : b * P + bs, :], stage[:bs, :])

    nc.compile()
    return nc


_CACHE = {}


def _prepare(u_f, v_f, adj_rows, adj_cols, adj_vals):
    """Host-side sharding: bucket edges by (core, superblock, window, block),
    pad (block, window) subcells to multiples of 128 (uniform across cores),
    and lay out per-core rows/vals/idx arrays in the SBUF tile layouts.

    Gather (idxs) order: superblock -> window -> block -> edges.
    Matmul (rows/vals) order: superblock -> block -> window -> edges.
    """
    rows = np.asarray(adj_rows, dtype=np.int64)
    cols = np.asarray(adj_cols, dtype=np.int64)
    vals = np.asarray(adj_vals, dtype=np.float32)

    core_of = rows // DESTS_PER_CORE
    blk_of = (rows % DESTS_PER_CORE) // P
    win_of = cols // WINDOW

    key = (core_of * N_BLOCKS + blk_of) * N_WINDOWS + win_of
    order = np.argsort(key, kind="stable")
    rows_s = rows[order]
    cols_s = cols[order]
    vals_s = vals[order]

    n_cells_total = N_CORES * N_BLOCKS * N_WINDOWS
    counts = np.bincount(key[order], minlength=n_cells_total).reshape(
        N_CORES, N_BLOCKS, N_WINDOWS
    )
    starts = np.zeros(n_cells_total + 1, dtype=np.int64)
    np.cumsum(counts.reshape(-1), out=starts[1:])

    max_counts = counts.max(axis=0)  # [N_BLOCKS, N_WINDOWS]
    cell_sizes = (np.ceil(max_counts / P).astype(np.int64) * P).tolist()
    for b in range(N_BLOCKS):
        if sum(cell_sizes[b]) == 0:
            cell_sizes[b][0] = P  # keep PSUM written for empty blocks

    total_padded = sum(sum(cs) for cs in cell_sizes)
    nt_total = total_padded // P
    idx_cols_total = total_padded // 16
    sblocks = _superblocks()

    per_core = []
    for c in range(N_CORES):
        rows_t = np.zeros((P, nt_total), np.float16)
        vals_t = np.zeros((P, nt_total), np.float32)
        idxs_t = np.zeros((P, idx_cols_total), np.int16)
        tile_pos = 0
        idx_pos = 0

        def cell_edges(b, q):
            ci = (c * N_BLOCKS + b) * N_WINDOWS + q
            s = starts[ci]
            e = s + counts[c, b, q]
            return rows_s[s:e], cols_s[s:e], vals_s[s:e]

        for sb in sblocks:
            # gather (idxs) order: window -> block
            for q in range(N_WINDOWS):
                for b in sb:
                    cell_n = cell_sizes[b][q]
                    if cell_n == 0:
                        continue
                    _, cc, _ = cell_edges(b, q)
                    ii = np.zeros(cell_n, np.int16)
                    ii[: len(cc)] = (cc - q * WINDOW).astype(np.int16)
                    wrapped = ii.reshape(cell_n // 16, 16).T
                    for g in range(8):
                        idxs_t[
                            16 * g : 16 * (g + 1), idx_pos : idx_pos + cell_n // 16
                        ] = wrapped
                    idx_pos += cell_n // 16
            # matmul (rows/vals) order: block -> window
            for b in sb:
                for q in range(N_WINDOWS):
                    cell_n = cell_sizes[b][q]
                    if cell_n == 0:
                        continue
                    rr, _, vv = cell_edges(b, q)
                    r = np.zeros(cell_n, np.float32)
                    v = np.zeros(cell_n, np.float32)
                    r[: len(rr)] = (rr - c * DESTS_PER_CORE - b * P).astype(np.float32)
                    v[: len(vv)] = vv
                    nt = cell_n // P
                    rows_t[:, tile_pos : tile_pos + nt] = r.reshape(nt, P).T
                    vals_t[:, tile_pos : tile_pos + nt] = v.reshape(nt, P).T
                    tile_pos += nt
        per_core.append((rows_t, vals_t, idxs_t))

    x_full = np.concatenate(
        [np.asarray(u_f, np.float32), np.asarray(v_f, np.float32)], axis=0
    )
    iota = np.broadcast_to(np.arange(P, dtype=np.float16), (P, P)).copy()
    in_maps = []
    for c in range(N_CORES):
        rows_t, vals_t, idxs_t = per_core[c]
        in_maps.append(
            {
                "x_t": np.ascontiguousarray(
                    x_full[c * DESTS_PER_CORE : (c + 1) * DESTS_PER_CORE, :].T
                ),
                "w": None,  # filled by caller
                "rows": rows_t,
                "vals": vals_t,
                "idxs": idxs_t,
                "iota": iota,
            }
        )
    return cell_sizes, nt_total, idx_cols_total, in_maps


def kernel(u_f, v_f, adj_rows, adj_cols, adj_vals, weight):
    w = np.asarray(weight, np.float32)
    cell_sizes, nt_total, idx_cols_total, in_maps = _prepare(
        u_f, v_f, adj_rows, adj_cols, adj_vals
    )
    for m in in_maps:
        m["w"] = w

    cache_key = (nt_total, idx_cols_total, tuple(tuple(cs) for cs in cell_sizes))
    if cache_key not in _CACHE:
        _CACHE.clear()
        _CACHE[cache_key] = _build_program(cell_sizes, nt_total, idx_cols_total)
    nc = _CACHE[cache_key]

    # The axon-tunneled device occasionally reports a transient
    # NRT_EXEC_UNIT_UNRECOVERABLE from a previous crashed run; a retry runs
    # on the freshly-reset device.
    last_err = None
    for _ in range(4):
        try:
            res = run_bass_kernel_spmd(nc, in_maps, core_ids=list(range(N_CORES)))
            break
        except Exception as e:  # noqa: BLE001
            last_err = e
    else:
        raise last_err
    return np.concatenate([res.results[c]["out"] for c in range(N_CORES)], axis=0)
